# revision 16
# baseline (speedup 1.0000x reference)
"""Trainium2 Bass kernel: multi-head attention (B=2, T=2048, C=2048, H=16, D=128).

Sharding: tensor-parallel over heads. 8 cores x 2 heads each; each core
computes a partial output, host sums the 8 partials.

Phase-interleaved schedule (single emission stream, per-engine FIFOs):
  W1: proj(b0)                     PE-bound, double-buffered PSUM
  W2: attn(b0) chunks interleaved with proj(b1) rt-iters
      -> ScalarE exp of b0 hides entirely under proj(b1)'s PE work;
         proj PSUM single-buffered, epilogues covered by attn chunks
  W3: attn(b1) chunks interleaved with outproj(b0) row-blocks
  W4: outproj(b1)

Attention per chunk (h, kb): mm1 -> [128,512] fp32 PSUM; exp on ScalarE;
mm2 accumulates yT.  Softmax denominator: two independent accumulators
per head (DVE-owned and GpSimd-owned, so neither engine's FIFO chains
through the other), adds emitted one chunk late so they never head-of-
line block; last 4 e-chunks go through a short PE ones-chain, col-group
packed 4 rows per PSUM bank -> ONE batched DVE reciprocal per qt pair;
recip rows staged to partition 0 by GpSimd-triggered DMAs, broadcast,
normalized in place -- all spread across later emission slots.
"""

import math

import numpy as np

N_CORES = 8
B, T, C = 2, 2048, 2048
N_HEAD, D = 16, 128
HPC = N_HEAD // N_CORES
JC = HPC * D

RT = 512                         # q tile in attention
KB = 128                         # key block
V_DIRECT, G_DIRECT = 0, 3        # exp writes these accumulators directly
V_ADD = (1, 2, 4, 5, 7, 8, 10)   # DVE adds
G_ADD = (6, 9, 11)               # GpSimd adds
TAILS = (12, 13, 14, 15)         # via PE ones-chain

PHASE_MARKS = []


def _build(Bp, Tp, Cp, hpc, d):
    PHASE_MARKS.clear()
    import concourse.bacc as bacc
    import concourse.tile as tile
    from concourse import mybir

    f32 = mybir.dt.float32
    f32r = mybir.dt.float32r
    bf16 = mybir.dt.bfloat16
    Exp = mybir.ActivationFunctionType.Exp
    Copy = mybir.ActivationFunctionType.Copy

    jc = hpc * d
    BT = Bp * Tp
    n_ck = Cp // 128
    n_kb = Tp // KB
    n_qt = Tp // RT
    n_rb = Tp // 128
    n_ot = Cp // RT
    scale = 1.0 / math.sqrt(d)
    RP = 256
    n_sub = RP // 128

    nc = bacc.Bacc("TRN2", target_bir_lowering=False, debug=False)

    xTp = nc.declare_dram_parameter("xTp", [128, n_ck, BT], bf16,
                                    isOutput=False)
    wqkv = nc.declare_dram_parameter("wqkv", [128, n_ck, 3 * jc], bf16,
                                     isOutput=False)
    wp = nc.declare_dram_parameter("wp", [jc, Cp], bf16, isOutput=False)
    ones_d = nc.declare_dram_parameter("ones", [128, 128], f32r, isOutput=False)
    cosT = nc.declare_dram_parameter("cosT", [d, Tp], bf16, isOutput=False)
    sinT = nc.declare_dram_parameter("sinT", [d, Tp], bf16, isOutput=False)
    # out[p, rb, :] = full_out[rb * 128 + p, :]  (host unpacks)
    out = nc.declare_dram_parameter("out", [128, BT // 128, Cp], bf16,
                                    isOutput=True)

    with tile.TileContext(nc) as tc:
        with (
            nc.allow_low_precision(reason="bf16 paths validated against the "
                                   "fp32 reference"),
            tc.tile_pool(name="wpool", bufs=1) as wpool,
            tc.tile_pool(name="acts", bufs=2) as acts,
            tc.tile_pool(name="xpool", bufs=1) as xpool,
            tc.tile_pool(name="rope", bufs=2) as rope,
            tc.tile_pool(name="epool", bufs=10) as epool,
            tc.tile_pool(name="dpool", bufs=2) as dpool,
            tc.tile_pool(name="small", bufs=2) as small,
            tc.tile_pool(name="bcpool", bufs=2) as bcpool,
            tc.tile_pool(name="opool", bufs=2) as opool,
        ):
            # ---- resident weights / tables / first xT batch ----
            TH = Tp // 2
            w_all = wpool.tile([128, n_ck, 3 * jc], bf16, tag="w")
            xt_tiles = {0: xpool.tile([128, n_ck, Tp], bf16, tag="xt",
                                      name="xt_b0")}

            def load_xt(b):
                for half in range(2):
                    t0 = half * TH
                    for ck in range(n_ck):
                        nc.sync.dma_start(
                            xt_tiles[b][:, ck, t0:t0 + TH],
                            xTp[:, ck, b * Tp + t0:b * Tp + t0 + TH])

            nc.sync.dma_start(w_all[:, 0:1, :], wqkv[:, 0:1, :])
            nc.sync.dma_start(xt_tiles[0][:, 0, 0:TH], xTp[:, 0, 0:TH])
            nc.sync.dma_start(w_all[:, 1:4, :], wqkv[:, 1:4, :])
            for ck in range(1, n_ck):
                nc.sync.dma_start(xt_tiles[0][:, ck, 0:TH],
                                  xTp[:, ck, 0:TH])
                if ck == 4:
                    nc.sync.dma_start(w_all[:, 4:10, :], wqkv[:, 4:10, :])
                if ck == 10:
                    nc.sync.dma_start(w_all[:, 10:16, :], wqkv[:, 10:16, :])
            for ck in range(n_ck):
                nc.sync.dma_start(xt_tiles[0][:, ck, TH:Tp],
                                  xTp[:, ck, TH:Tp])
            cos_sb = wpool.tile([d, Tp], bf16, tag="cos")
            sin_sb = wpool.tile([d, Tp], bf16, tag="sin")
            nc.sync.dma_start(cos_sb, cosT[:])
            nc.sync.dma_start(sin_sb, sinT[:])
            ones_sb = wpool.tile([128, 1], f32r, tag="ones")
            nc.sync.dma_start(ones_sb, ones_d[:, 0:1])
            ones_bf = wpool.tile([128, 1], bf16, tag="ones_bf")
            nc.vector.tensor_copy(out=ones_bf, in_=ones_sb)
            wp_sb = wpool.tile([128, hpc, Cp], bf16, tag="wp")

            def wqs(ck, h):
                return w_all[:, ck, h * d:(h + 1) * d]

            def wks(ck, h):
                return w_all[:, ck, jc + h * d:jc + (h + 1) * d]

            def wvs(ck):
                return w_all[:, ck, 2 * jc:3 * jc]

            # per-batch activation tiles (bufs=2: b and b+1 coexist)
            bt = {}

            def batch_tiles(b):
                if b not in bt:
                    bt[b] = dict(
                        qT=acts.tile([128, hpc, Tp], bf16, tag="qT",
                                     name=f"qT{b}"),
                        kT=acts.tile([128, hpc, Tp], bf16, tag="kT",
                                     name=f"kT{b}"),
                        v=acts.tile([128, n_kb, jc], bf16, tag="v",
                                    name=f"v{b}"),
                        yT=acts.tile([128, hpc, Tp], bf16, tag="yT",
                                     name=f"yT{b}"),
                    )
                return bt[b]

            def emit_proj_rt(b, rt, psp, vcopy_dve):
                t = batch_tiles(b)
                xt_b = xt_tiles[b]
                tsl = slice(rt * RP, (rt + 1) * RP)
                q_ps = psp.tile([128, hpc * RP], f32, tag="qps")
                k_ps = psp.tile([128, hpc * RP], f32, tag="kps")
                v_ps = psp.tile([128, n_sub * jc], f32, tag="vps", bufs=1)
                for ck in range(n_ck):
                    xt = xt_b[:, ck, tsl]
                    first, last = ck == 0, ck == n_ck - 1
                    for h in range(hpc):
                        nc.tensor.matmul(
                            q_ps[:, h * RP:(h + 1) * RP], wqs(ck, h), xt,
                            start=(first and h == 0),
                            stop=(last and h == hpc - 1),
                            skip_group_check=True)
                        nc.tensor.matmul(
                            k_ps[:, h * RP:(h + 1) * RP], wks(ck, h), xt,
                            start=(first and h == 0),
                            stop=(last and h == hpc - 1),
                            skip_group_check=True)
                    for s in range(n_sub):
                        nc.tensor.matmul(
                            v_ps[:, s * jc:(s + 1) * jc],
                            xt[:, s * 128:(s + 1) * 128], wvs(ck),
                            start=(first and s == 0),
                            stop=(last and s == n_sub - 1),
                            skip_group_check=True)
                hd = d // 2
                for h in range(hpc):
                    for ps, dst in (
                        (q_ps[:, h * RP:(h + 1) * RP], t["qT"]),
                        (k_ps[:, h * RP:(h + 1) * RP], t["kT"]),
                    ):
                        t1 = rope.tile([d, RP], f32, tag="t1")
                        nc.vector.tensor_mul(t1, ps, cos_sb[:, tsl])
                        t2 = rope.tile([d, RP], f32, tag="t2")
                        nc.vector.tensor_mul(
                            t2[0:hd], ps[hd:d], sin_sb[0:hd, tsl])
                        nc.vector.tensor_mul(
                            t2[hd:d], ps[0:hd], sin_sb[hd:d, tsl])
                        nc.vector.tensor_add(dst[:, h, tsl], t1, t2)
                for s in range(n_sub):
                    dst = t["v"][:, rt * n_sub + s, :]
                    src = v_ps[:, s * jc:(s + 1) * jc]
                    if vcopy_dve:
                        nc.vector.tensor_copy(out=dst, in_=src)
                    else:
                        nc.scalar.activation(dst, src, Copy)

            def make_attn_stepper(b, ps_s, ps_y, ps_d, fin_cell):
                """Generator emitting attention for batch b, one (h, kb)
                chunk per next().  fin_cell[0] carries a leftover finalize
                generator for the caller to keep advancing."""
                t = batch_tiles(b)
                qT_sb, kT_sb, v_sb, yT_sb = (t["qT"], t["kT"], t["v"],
                                             t["yT"])

                def mm1(qt, j, h):
                    qsl = slice(qt * RT, (qt + 1) * RT)
                    s_ps = ps_s.tile([128, RT], f32, tag="s",
                                     name=f"s{b}_{qt}_{j}_{h}")
                    nc.tensor.matmul(
                        s_ps, kT_sb[:, h, j * KB:(j + 1) * KB],
                        qT_sb[:, h, qsl], start=True, stop=True,
                        skip_group_check=True)
                    return s_ps

                def _emit_norm(qt, h, bc):
                    qsl = slice(qt * RT, (qt + 1) * RT)
                    nc.vector.tensor_mul(
                        yT_sb[:, h, qsl], yT_sb[:, h, qsl], bc)

                def finalize_ops(qt_pair, dsum_ps):
                    r_sb = small.tile([128, RT], bf16, tag="recip",
                                      name=f"r{b}_{qt_pair}")
                    nc.vector.reciprocal(r_sb[0:97, :], dsum_ps[0:97, :])
                    yield
                    pending = []
                    for qt in (2 * qt_pair, 2 * qt_pair + 1):
                        for h in range(hpc):
                            g = 32 * ((qt % 2) * hpc + h)
                            st = bcpool.tile([1, RT], bf16, tag="stage",
                                             name=f"st{b}_{qt}_{h}")
                            nc.gpsimd.dma_start(st[0:1, :], r_sb[g:g + 1, :])
                            bc = bcpool.tile([128, RT], bf16, tag="bc",
                                             name=f"bc{b}_{qt}_{h}")
                            nc.gpsimd.partition_broadcast(
                                out_ap=bc, in_ap=st[0:1, :])
                            pending.append((qt, h, bc))
                            yield
                            if len(pending) >= 2:
                                _emit_norm(*pending.pop(0))
                                yield
                    while pending:
                        _emit_norm(*pending.pop(0))
                        yield

                def gen():
                    state = {}

                    def start_qt(qt):
                        state[qt] = dict(
                            ys=[ps_y.tile([d, RT], f32, tag="y",
                                          name=f"y{b}_{qt}_{h}")
                                for h in range(hpc)],
                            dv=[None, None], dg=[None, None],
                            tails=[[], []])

                    def chunk_of(c):
                        qt, r = divmod(c, 2 * n_kb)
                        j, h = divmod(r, hpc)
                        return qt, j, h

                    start_qt(0)
                    n_chunks = n_qt * n_kb * hpc
                    LOOK = 2
                    pend = []
                    for c in range(LOOK):
                        pend.append(mm1(*chunk_of(c)))
                    adds = []          # delayed (engine, acc, e) ops
                    fin_gen = None
                    dsum_ps = None
                    for c in range(n_chunks):
                        qt, j, h = chunk_of(c)
                        if j == 0 and h == 0 and qt % 2 == 0:
                            dsum_ps = ps_d.tile([128, RT], f32, tag="dsum",
                                                name=f"ds{b}_{qt // 2}")
                        st = state[qt]
                        s_ps = pend.pop(0)
                        # exp
                        if j == V_DIRECT:
                            e = dpool.tile([128, RT], bf16, tag=f"dv{h}",
                                           name=f"dv{b}_{qt}_{h}")
                            st["dv"][h] = e
                        elif j == G_DIRECT:
                            e = dpool.tile([128, RT], bf16, tag=f"dg{h}",
                                           name=f"dg{b}_{qt}_{h}")
                            st["dg"][h] = e
                        else:
                            e = epool.tile([128, RT], bf16, tag="e",
                                           name=f"e{b}_{qt}_{j}_{h}")
                        nc.scalar.activation(e, s_ps, Exp, scale=scale)
                        # mm2
                        nc.tensor.matmul(
                            st["ys"][h], v_sb[:, j, h * d:(h + 1) * d], e,
                            start=(j == 0), stop=(j == n_kb - 1),
                            skip_group_check=True)
                        # queue the dacc op (emitted 2 chunks later)
                        if j in V_ADD:
                            adds.append(("v", st["dv"][h], e))
                        elif j in G_ADD:
                            adds.append(("g", st["dg"][h], e))
                        elif j in TAILS:
                            st["tails"][h].append(e)
                        if len(adds) > 2:
                            kind, acc, ee = adds.pop(0)
                            if kind == "v":
                                nc.vector.tensor_add(acc, acc, ee)
                            else:
                                nc.gpsimd.tensor_add(acc, acc, ee)
                        # advance finalize
                        if fin_gen is not None and (j, h) >= (1, 0):
                            if next(fin_gen, StopIteration) is StopIteration:
                                fin_gen = None
                        # mm1 lookahead
                        nc_ = c + LOOK
                        if nc_ < n_chunks:
                            nqt = chunk_of(nc_)[0]
                            if nqt not in state:
                                start_qt(nqt)
                            pend.append(mm1(*chunk_of(nc_)))
                        # qt boundary bookkeeping
                        if j == n_kb - 1 and h == hpc - 1:
                            while adds:
                                kind, acc, ee = adds.pop(0)
                                if kind == "v":
                                    nc.vector.tensor_add(acc, acc, ee)
                                else:
                                    nc.gpsimd.tensor_add(acc, acc, ee)
                            for hh in range(hpc):
                                g = 32 * ((qt % 2) * hpc + hh)
                                terms = ([st["dv"][hh], st["dg"][hh]]
                                         + st["tails"][hh])
                                for i, tm in enumerate(terms):
                                    nc.tensor.matmul(
                                        dsum_ps[g:g + 1, :], ones_bf, tm,
                                        start=(i == 0),
                                        stop=(i == len(terms) - 1),
                                        skip_group_check=True,
                                        tile_position=(0, g))
                            qsl = slice(qt * RT, (qt + 1) * RT)
                            for hh in range(hpc):
                                nc.vector.tensor_copy(
                                    out=yT_sb[:, hh, qsl],
                                    in_=st["ys"][hh])
                            del state[qt]
                            if qt % 2 == 1:
                                if fin_gen is not None:
                                    for _ in fin_gen:
                                        pass
                                fin_gen = finalize_ops(qt // 2, dsum_ps)
                                next(fin_gen)
                        yield
                    fin_cell[0] = fin_gen
                return gen()

            def emit_outproj_rb(b, rb, ps_o, fin_cell, copy_split):
                t = batch_tiles(b)
                o_sb = opool.tile([128, Cp], bf16, tag="o",
                                  name=f"o{b}_{rb}")
                for ot in range(n_ot):
                    o_ps = ps_o.tile([128, RT], f32, tag="ops")
                    for h in range(hpc):
                        nc.tensor.matmul(
                            o_ps, t["yT"][:, h, rb * 128:(rb + 1) * 128],
                            wp_sb[:, h, ot * RT:(ot + 1) * RT],
                            start=(h == 0), stop=(h == hpc - 1))
                    osl = o_sb[:, ot * RT:(ot + 1) * RT]
                    if copy_split and ot % 2 == 1:
                        nc.scalar.activation(osl, o_ps, Copy)
                    else:
                        nc.vector.tensor_copy(out=osl, in_=o_ps)
                    if fin_cell[0] is not None:
                        if next(fin_cell[0], StopIteration) is StopIteration:
                            fin_cell[0] = None
                nc.sync.dma_start(out[:, b * (Tp // 128) + rb, :], o_sb)

            # ================= W1: proj b0 =================
            PHASE_MARKS.append(("proj0", nc.next_id()))
            with tc.tile_pool(name="psp0", bufs=2, space="PSUM") as psp0:
                for rt in range(Tp // RP):
                    emit_proj_rt(0, rt, psp0, vcopy_dve=False)

            # ============ W2: attn b0 || proj b1 ============
            PHASE_MARKS.append(("w2", nc.next_id()))
            nc.sync.dma_start(wp_sb, wp.rearrange("(h p) o -> p h o", p=128))
            xt_tiles[1] = xpool.tile([128, n_ck, Tp], bf16, tag="xt",
                                     name="xt_b1")
            load_xt(1)
            fin0 = [None]
            with (
                tc.tile_pool(name="psp1", bufs=1, space="PSUM") as psp1,
                tc.tile_pool(name="s0", bufs=2, space="PSUM") as s0,
                tc.tile_pool(name="y0", bufs=2, space="PSUM") as y0,
                tc.tile_pool(name="d0", bufs=1, space="PSUM") as d0,
            ):
                stepper0 = make_attn_stepper(0, s0, y0, d0, fin0)
                n_chunks = n_qt * n_kb * hpc
                per_rt = n_chunks // (Tp // RP)
                for rt in range(Tp // RP):
                    for _ in range(per_rt):
                        next(stepper0)
                    emit_proj_rt(1, rt, psp1, vcopy_dve=True)
                for _ in stepper0:
                    pass

            # ========== W3: attn b1 || outproj b0 ==========
            PHASE_MARKS.append(("w3", nc.next_id()))
            fin1 = [None]
            with (
                tc.tile_pool(name="s1", bufs=3, space="PSUM") as s1,
                tc.tile_pool(name="y1", bufs=2, space="PSUM") as y1,
                tc.tile_pool(name="d1", bufs=1, space="PSUM") as d1,
                tc.tile_pool(name="po0", bufs=2, space="PSUM") as po0,
            ):
                stepper1 = make_attn_stepper(1, s1, y1, d1, fin1)
                n_chunks = n_qt * n_kb * hpc
                per_rb = n_chunks // n_rb
                for rb in range(n_rb):
                    for _ in range(per_rb):
                        next(stepper1)
                    if fin0[0] is not None:
                        if next(fin0[0], StopIteration) is StopIteration:
                            fin0[0] = None
                    emit_outproj_rb(0, rb, po0, fin0, copy_split=False)
                for _ in stepper1:
                    pass

            # ================= W4: outproj b1 =================
            PHASE_MARKS.append(("outproj1", nc.next_id()))
            with tc.tile_pool(name="po1", bufs=6, space="PSUM") as po1:
                for rb in range(n_rb):
                    emit_outproj_rb(1, rb, po1, fin1, copy_split=True)

    PHASE_MARKS.append(("tail", nc.next_id()))
    nc.compile()
    return nc


def _prep_in_maps(x, cos, sin, W_qkv, W_proj, n_cores, hpc, d):
    """Host-side shard prep: pure layout work (transpose / slice / pack)."""
    Bp, Tp, Cp = x.shape
    jc = hpc * d
    n_ck = Cp // 128
    import ml_dtypes
    xT = np.ascontiguousarray(x.reshape(Bp * Tp, Cp).T)
    xTp = np.ascontiguousarray(
        xT.reshape(n_ck, 128, Bp * Tp).transpose(1, 0, 2)
    ).astype(ml_dtypes.bfloat16)
    cosT = np.ascontiguousarray(cos.T).astype(ml_dtypes.bfloat16)
    sinT = np.ascontiguousarray(sin.T).copy()
    sinT[: d // 2] *= -1.0
    sinT = sinT.astype(ml_dtypes.bfloat16)
    in_maps = []
    for c in range(n_cores):
        j0, j1 = c * jc, (c + 1) * jc
        wcat = np.concatenate(
            [W_qkv[:, j0:j1], W_qkv[:, Cp + j0:Cp + j1],
             W_qkv[:, 2 * Cp + j0:2 * Cp + j1]], axis=1)
        wpk = np.ascontiguousarray(
            wcat.reshape(n_ck, 128, 3 * jc).transpose(1, 0, 2)
        ).astype(ml_dtypes.bfloat16)
        in_maps.append({
            "xTp": xTp,
            "wqkv": wpk,
            "wp": np.ascontiguousarray(W_proj[j0:j1, :]).astype(ml_dtypes.bfloat16),
            "ones": np.ones((128, 128), dtype=np.float32),
            "cosT": cosT,
            "sinT": sinT,
        })
    return in_maps


def _install_ntff_hook():
    """Enable NTFF profiling under axon when the boot image lacks the
    antenv.axon_hooks shim. Harmless if anything is missing."""
    import sys
    import types
    try:
        from antenv.axon_hooks import get_axon_ntff_profile_hook
        if get_axon_ntff_profile_hook() is not None:
            return
    except ImportError:
        pass
    try:
        sys.path.insert(0, "/root/.axon_site")
        from trn_agent_boot.trn_boot import _ntff_profile_via_ctypes

        hook = _ntff_profile_via_ctypes("/opt/axon/libaxon_pjrt.so")
        if hook is None:
            return
        mod = types.ModuleType("antenv.axon_hooks")
        mod.get_axon_ntff_profile_hook = lambda: hook
        mod.set_axon_ntff_profile_hook = lambda h: None
        import antenv
        antenv.axon_hooks = mod
        sys.modules["antenv.axon_hooks"] = mod
    except Exception:
        pass


def _run(x, cos, sin, W_qkv, W_proj, trace=False):
    from concourse.bass_utils import run_bass_kernel_spmd

    if trace:
        _install_ntff_hook()

    x = np.ascontiguousarray(x, dtype=np.float32)
    cos = np.ascontiguousarray(cos, dtype=np.float32)
    sin = np.ascontiguousarray(sin, dtype=np.float32)
    W_qkv = np.ascontiguousarray(W_qkv, dtype=np.float32)
    W_proj = np.ascontiguousarray(W_proj, dtype=np.float32)

    Bp, Tp, Cp = x.shape
    nc = _build(Bp, Tp, Cp, HPC, D)
    in_maps = _prep_in_maps(x, cos, sin, W_qkv, W_proj, N_CORES, HPC, D)
    res = run_bass_kernel_spmd(nc, in_maps, core_ids=list(range(N_CORES)),
                               trace=trace)
    acc = np.zeros((Bp * Tp, Cp), dtype=np.float32)
    for i in range(N_CORES):
        o = np.asarray(res.results[i]["out"], dtype=np.float32)
        acc += o.transpose(1, 0, 2).reshape(Bp * Tp, Cp)
    return acc.reshape(Bp, Tp, Cp), res


def kernel(x, cos, sin, W_qkv, W_proj):
    out, _ = _run(x, cos, sin, W_qkv, W_proj, trace=False)
    return out
